# revision 9
# baseline (speedup 1.0000x reference)
"""DeepSeek-style MLA decode attention (batch=8, 128 heads, cache 512) on 8 NeuronCores.

Sharding: tensor-parallel over heads (16 heads/core).
 - Down-projection row-sharded over HID: core c computes a partial
   c = x_slice @ [Wq_down | Wkv_down]_slice; tiny AllReduce (64 KB) gives every
   core the full latent c = [c_q (1536) | c_kv (512)].
 - Wq_up / Wv_up column-sharded by head: each core computes q / v_new for its
   own 16 heads directly (no big collective on the q path).
 - k_cache / v_cache sharded by head, host-pretransposed, stored fp8e4m3
   (fp8 moving operands stream 2 elem/cycle on the PE).
 - Weights and matmul operands in bf16 (PSUM accumulation stays fp32).
 - o_proj input rows sharded by head; partial outputs ReduceScattered (bf16,
   3 overlapping chunks) over the batch dim; core b returns batch b's row.
 - A dummy collective at t=0 absorbs cross-core start skew so the real
   AllReduce doesn't eat the ~37us barrier on the critical path.

Note: the reference's "new token" softmax is over a length-1 axis (== 1.0), so
k_new/Wk_up are dead and the new-token contribution is simply + v_new.
"""

import numpy as np
import ml_dtypes

import concourse.bass as bass
import concourse.mybir as mybir
import concourse.tile as tile
from concourse import bacc
from concourse import bass_utils
from concourse.masks import make_identity

NC_ = 8                      # cores
B = 8                        # batch
H = 128                      # total heads
HP = H // NC_                # 16 heads per core
D = 128                      # head dim
L = 512                      # cache len
HID = 7168
CH = HID // NC_              # 896 hid rows per core (7 chunks of 128)
QL = 1536
KVL = 512
CL = QL + KVL                # 2048 latent dims
NH = HP * D                  # 2048 per-core head cols
SCALE = 1.0 / float(np.sqrt(D))
F32 = mybir.dt.float32
BF16 = mybir.dt.bfloat16
F8 = mybir.dt.float8e4

BF_NP = ml_dtypes.bfloat16
F8_NP = ml_dtypes.float8_e4m3

# o_proj rounds: (col0, col1, n accumulators of 512)
O_ROUNDS = ((0, 3072, 6), (3072, 6144, 6), (6144, 7168, 2))


def build_nc():
    nc = bacc.Bacc(
        "TRN2",
        target_bir_lowering=False,
        debug=False,
        enable_asserts=True,
        num_devices=NC_,
    )
    xt = nc.dram_tensor("xt", [CH, B], BF16, kind="ExternalInput").ap()
    w_down = nc.dram_tensor("w_down", [CH, CL], BF16, kind="ExternalInput").ap()
    wq_up = nc.dram_tensor("wq_up", [QL, NH], BF16, kind="ExternalInput").ap()
    wv_up = nc.dram_tensor("wv_up", [KVL, NH], BF16, kind="ExternalInput").ap()
    kt = nc.dram_tensor("kt", [8, 128, 8192], F8, kind="ExternalInput").ap()
    v = nc.dram_tensor("v", [8, 128, 8192], F8, kind="ExternalInput").ap()
    wo = nc.dram_tensor("wo", [NH, HID], BF16, kind="ExternalInput").ap()
    o = nc.dram_tensor("o", [1, HID], F32, kind="ExternalOutput").ap()

    rg = [list(range(NC_))]

    with tile.TileContext(nc) as tc:
        with (
            tc.tile_pool(name="const", bufs=1) as constp,
            tc.tile_pool(name="sbuf", bufs=1) as sb,
            tc.tile_pool(name="stage", bufs=2) as stg,
            tc.tile_pool(name="ktp", bufs=2) as ktp,
            tc.tile_pool(name="vp", bufs=2) as vp,
            tc.tile_pool(name="wop", bufs=2) as wop,
            tc.tile_pool(name="psbank", bufs=8, space="PSUM") as psbank,
            tc.tile_pool(name="dram", bufs=1, space="DRAM") as dram,
        ):
            ident = constp.tile([128, 128], F32)
            make_identity(nc, ident[:])
            id8 = ident[0:8, 0:8]
            # uint8 one-hot columns for CopyPredicated masks (must be int dtype)
            identu8 = constp.tile([128, 128], mybir.dt.uint8, tag="identu8")
            nc.vector.tensor_copy(identu8[:], ident[:])

            # Dummy collective: absorbs cross-core NEFF start skew off the
            # critical path (runs while the weight DMAs stream in).
            dummy = stg.tile([1, 8], F32, tag="dummy")
            nc.gpsimd.memset(dummy[:], 0.0)
            dummy_b = dram.tile([1, 8], F32, tag="dummyb")
            nc.sync.dma_start(out=dummy_b[:], in_=dummy[:])
            dummy_r = dram.tile([1, 8], F32, tag="dummyr")
            nc.gpsimd.collective_compute(
                "AllReduce",
                mybir.AluOpType.add,
                replica_groups=rg,
                ins=[dummy_b.opt()],
                outs=[dummy_r.opt()],
            )

            # ---------- partial latent: c_part = x_slice @ W_down_slice ----------
            xt_sb = constp.tile([128, 7, B], BF16, tag="xt")
            nc.sync.dma_start(
                out=xt_sb[:], in_=xt.rearrange("(c p) b -> p c b", p=128)
            )
            wd_sb = constp.tile([128, 7, CL], BF16, tag="wd")
            for i in range(7):
                nc.sync.dma_start(
                    out=wd_sb[:, i, :], in_=w_down[i * 128:(i + 1) * 128, :]
                )
            c_part = sb.tile([B, CL], F32, tag="cpart")
            ps_cds = [
                psbank.tile([B, 512], F32, tag="bank", name=f"ps_cd{n}")
                for n in range(4)
            ]
            for i in range(7):
                for n in range(4):
                    nc.tensor.matmul(
                        ps_cds[n][:B, :],
                        xt_sb[:, i, :],
                        wd_sb[:, i, n * 512:(n + 1) * 512],
                        start=(i == 0), stop=(i == 6),
                    )
            for n in range(4):
                nc.vector.tensor_copy(
                    c_part[:, n * 512:(n + 1) * 512], ps_cds[n][:B, :]
                )

            c_bounce = dram.tile([B, CL], F32, tag="cb")
            nc.sync.dma_start(out=c_bounce[:], in_=c_part[:])
            c_red = dram.tile([B, CL], F32, tag="cr")
            nc.gpsimd.collective_compute(
                "AllReduce",
                mybir.AluOpType.add,
                replica_groups=rg,
                ins=[c_bounce.opt()],
                outs=[c_red.opt()],
            )
            c_all = sb.tile([B, CL], F32, tag="call")
            nc.sync.dma_start(out=c_all[:], in_=c_red[:])

            # cT [128, 16*8]: rank-chunk j on partitions, batch on free
            ps_cT = psbank.tile([128, 512], F32, tag="bank")
            for j in range(16):
                nc.tensor.transpose(
                    ps_cT[0:128, j * 8:(j + 1) * 8],
                    c_all[:, j * 128:(j + 1) * 128],
                    id8,
                )
            cT = sb.tile([128, 128], BF16, tag="cT")
            nc.vector.tensor_copy(cT[:], ps_cT[:, 0:128])

            # ---------- q_own = c_q @ Wq_up_c ; vnew = c_kv @ Wv_up_c ----------
            wqup_sb = constp.tile([128, 12, NH], BF16, tag="wqup")
            for s in range(3):
                nc.sync.dma_start(
                    out=wqup_sb[:, s * 4:(s + 1) * 4, :],
                    in_=wq_up[s * 512:(s + 1) * 512, :].rearrange(
                        "(c p) n -> p c n", p=128
                    ),
                )
            wvup_sb = constp.tile([128, 4, NH], BF16, tag="wvup")
            nc.sync.dma_start(
                out=wvup_sb[:], in_=wv_up.rearrange("(c p) n -> p c n", p=128)
            )
            qown = sb.tile([B, NH], F32, tag="qown")
            vnew = sb.tile([B, NH], F32, tag="vnew")
            for n in range(4):
                ps_q = psbank.tile([B, 512], F32, tag="bank")
                for j in range(12):
                    nc.tensor.matmul(
                        ps_q[:B, :],
                        cT[:, j * 8:(j + 1) * 8],
                        wqup_sb[:, j, n * 512:(n + 1) * 512],
                        start=(j == 0), stop=(j == 11),
                    )
                nc.vector.tensor_copy(qown[:, n * 512:(n + 1) * 512], ps_q[:B, :])
                ps_vn = psbank.tile([B, 512], F32, tag="bank")
                for j in range(4):
                    nc.tensor.matmul(
                        ps_vn[:B, :],
                        cT[:, (12 + j) * 8:(13 + j) * 8],
                        wvup_sb[:, j, n * 512:(n + 1) * 512],
                        start=(j == 0), stop=(j == 3),
                    )
                nc.vector.tensor_copy(vnew[:, n * 512:(n + 1) * 512], ps_vn[:B, :])

            # qT [128 d, 128 hb] (hb = h*8+b), bf16 for the score matmuls
            ps_qT = psbank.tile([128, 512], F32, tag="bank")
            for h in range(HP):
                nc.tensor.transpose(
                    ps_qT[0:128, h * 8:(h + 1) * 8],
                    qown[:, h * D:(h + 1) * D],
                    id8,
                )
            qT = sb.tile([128, 128], BF16, tag="qT")
            nc.vector.tensor_copy(qT[:], ps_qT[:, 0:128])

            # ---------------- phase A: scores over k cache ----------------
            # lhsT = qT (bf16, stationary); rhs = fp8 kT tile (moving, N=512).
            # Out row hb of each product is the valid score row; extract it
            # (bf16) with a partition-aligned predicated copy on the DVE.
            scores = sb.tile([128, 512], BF16, tag="scores")
            for t8 in range(8):
                kt_t = ktp.tile([128, 8192], F8, tag="kt")
                nc.sync.dma_start(out=kt_t[:], in_=kt[t8])
                for u in range(16):
                    hb = 16 * t8 + u
                    ps_s = psbank.tile([128, 512], F32, tag="bank")
                    nc.tensor.matmul(
                        ps_s[:], qT[:], kt_t[:, u * 512:(u + 1) * 512],
                        start=True, stop=True,
                    )
                    # write only row hb (engines can't address partition hb
                    # directly: start partition must be 0/32/64/96)
                    nc.vector.copy_predicated(
                        scores[:],
                        identu8[:, hb:hb + 1].broadcast_to((128, 512)),
                        ps_s[:],
                    )

            probs = sb.tile([128, 512], F32, tag="probs")
            denom = sb.tile([128, 1], F32, tag="denom")
            nc.scalar.activation(
                probs[:], scores[:], mybir.ActivationFunctionType.Exp,
                scale=SCALE, accum_out=denom[:],
            )
            recip = sb.tile([128, 1], F32, tag="recip")
            nc.vector.reciprocal(recip[:], denom[:])
            probsn = sb.tile([128, 512], F32, tag="probsn")
            nc.vector.tensor_scalar_mul(probsn[:], probs[:], recip[:])

            ps_pT = psbank.tile([128, 512], F32, tag="bank")
            for cc in range(4):
                nc.tensor.transpose(
                    ps_pT[:, cc * 128:(cc + 1) * 128],
                    probsn[:, cc * 128:(cc + 1) * 128],
                    ident[:],
                )
            probsT = sb.tile([128, 4, 128], BF16, tag="probsT")
            nc.vector.tensor_copy(
                probsT[:].rearrange("p c n -> p (c n)"), ps_pT[:]
            )

            # ---------------- phase B: attn rows = probs @ V ----------------
            # Per group of 4 hb: lhsT = probsT chunk cc (bf16, all hb), rhs
            # packs the 4 hb's fp8 V chunk cc; accumulate over cc, then
            # extract row 4g+u from column block u.
            attn = sb.tile([128, 128], F32, tag="attn")
            for g8 in range(8):
                v_t = vp.tile([128, 8192], F8, tag="v")
                nc.sync.dma_start(out=v_t[:], in_=v[g8])
                for gg in range(4):
                    g = 4 * g8 + gg
                    ps_a = psbank.tile([128, 512], F32, tag="bank")
                    for cc in range(4):
                        nc.tensor.matmul(
                            ps_a[:],
                            probsT[:, cc, :],
                            v_t[:, gg * 2048 + cc * 512:gg * 2048 + (cc + 1) * 512],
                            start=(cc == 0), stop=(cc == 3),
                        )
                    for u in range(4):
                        hb = 4 * g + u
                        nc.vector.copy_predicated(
                            attn[:],
                            identu8[:, hb:hb + 1].broadcast_to((128, 128)),
                            ps_a[:, u * 128:(u + 1) * 128],
                        )

            # vnewT, then attnT = attn^T + vnewT (bf16 operand for o_proj)
            ps_vT = psbank.tile([128, 512], F32, tag="bank")
            for h in range(HP):
                nc.tensor.transpose(
                    ps_vT[0:128, h * 8:(h + 1) * 8],
                    vnew[:, h * D:(h + 1) * D],
                    id8,
                )
            vnewT = sb.tile([128, 128], F32, tag="vnewT")
            nc.vector.tensor_copy(vnewT[:], ps_vT[:, 0:128])
            ps_aT = psbank.tile([128, 512], F32, tag="bank")
            nc.tensor.transpose(ps_aT[:, 0:128], attn[:], ident[:])
            attnT = sb.tile([128, 128], BF16, tag="attnT")
            nc.vector.tensor_add(attnT[:], ps_aT[:, 0:128], vnewT[:])

            # ---------------- phase C: o_part = attn^T @ Wo_c ----------------
            # Rounds of up to 6 n-chunks; Wo streams as per-head row blocks.
            # Partial outputs RS'd (bf16) per round, overlapping the next round.
            ors_parts = []
            for r, (n0, n1, nn) in enumerate(O_ROUNDS):
                ps_os = [
                    psbank.tile([B, 512], F32, tag="bank", name=f"ps_o{r}_{i}")
                    for i in range(nn)
                ]
                for h in range(HP):
                    wo_t = wop.tile([128, 3072], BF16, tag="wo")
                    nc.sync.dma_start(
                        out=wo_t[:, 0:n1 - n0],
                        in_=wo[h * D:(h + 1) * D, n0:n1],
                    )
                    for i in range(nn):
                        nc.tensor.matmul(
                            ps_os[i][:B, :],
                            attnT[:, h * 8:(h + 1) * 8],
                            wo_t[:, i * 512:(i + 1) * 512],
                            start=(h == 0), stop=(h == HP - 1),
                        )
                ostage = stg.tile([B, 3072], BF16, tag="ostage")
                for i in range(nn):
                    nc.vector.tensor_copy(
                        ostage[:, i * 512:(i + 1) * 512], ps_os[i][:B, :]
                    )
                ob_r = dram.tile([B, n1 - n0], BF16, tag=f"ob{r}", name=f"ob{r}")
                nc.sync.dma_start(out=ob_r[:], in_=ostage[:, 0:n1 - n0])
                or_r = dram.tile([1, n1 - n0], BF16, tag=f"or{r}", name=f"or{r}")
                nc.gpsimd.collective_compute(
                    "ReduceScatter",
                    mybir.AluOpType.add,
                    replica_groups=rg,
                    ins=[ob_r.opt()],
                    outs=[or_r.opt()],
                )
                ors_parts.append((or_r, n0, n1))

            for or_r, n0, n1 in ors_parts:
                fin_b = stg.tile([1, 3072], BF16, tag="finb", bufs=1)
                nc.sync.dma_start(out=fin_b[:, 0:n1 - n0], in_=or_r[:])
                for f0 in range(0, n1 - n0, 1536):
                    f1 = min(f0 + 1536, n1 - n0)
                    fin_f = stg.tile([1, 1536], F32, tag="finf", bufs=1)
                    nc.vector.tensor_copy(fin_f[:, 0:f1 - f0], fin_b[:, f0:f1])
                    nc.sync.dma_start(
                        out=o[:, n0 + f0:n0 + f1], in_=fin_f[:, 0:f1 - f0]
                    )

    nc.compile()
    return nc


_NC_CACHE = None


def _get_nc():
    global _NC_CACHE
    if _NC_CACHE is None:
        _NC_CACHE = build_nc()
    return _NC_CACHE


def make_in_maps(x, k_cache, v_cache, Wq_down, Wq_up, Wkv_down, Wv_up, Wo):
    xT = np.ascontiguousarray(np.asarray(x, np.float32).reshape(B, HID).T)
    wd_full = np.concatenate(
        [np.asarray(Wq_down, np.float32), np.asarray(Wkv_down, np.float32)], axis=1
    )
    k = np.asarray(k_cache, np.float32)
    v = np.asarray(v_cache, np.float32)
    wq_up = np.asarray(Wq_up, np.float32)
    wv_up = np.asarray(Wv_up, np.float32)
    wo = np.asarray(Wo, np.float32)
    in_maps = []
    for c in range(NC_):
        hs = slice(c * HP, (c + 1) * HP)
        rs = slice(c * CH, (c + 1) * CH)
        cs = slice(c * NH, (c + 1) * NH)
        # kt merged tile t8 holds hb=16*t8..16*t8+16 as [128 d, (u, keys)]
        kt_c = (
            k[:, hs]
            .transpose(1, 0, 3, 2)          # (16, 8, 128, 512) [h, b, d, l]
            .reshape(32, 4, 128, 512)       # [g, t, d, l]
            .transpose(0, 2, 1, 3)          # [g, d, t, l]
            .reshape(8, 4, 128, 2048)       # [t8, tt, d, (t l)]
            .transpose(0, 2, 1, 3)          # [t8, d, tt, (t l)]
            .reshape(8, 128, 8192)
        )
        # v merged tile g8 holds hb=16*g8..16*g8+16 as [128 lp, (gg, cc, t, d)]
        v_c = (
            v[:, hs]
            .transpose(1, 0, 2, 3)          # (16, 8, 512, 128) [h, b, l, d]
            .reshape(32, 4, 4, 128, 128)    # [g, t, cc, lp, d]
            .transpose(0, 3, 2, 1, 4)       # [g, lp, cc, t, d]
            .reshape(8, 4, 128, 2048)       # [g8, gg, lp, (cc t d)]
            .transpose(0, 2, 1, 3)          # [g8, lp, gg, (cc t d)]
            .reshape(8, 128, 8192)
        )
        in_maps.append(
            {
                "xt": np.ascontiguousarray(xT[rs]).astype(BF_NP),
                "w_down": np.ascontiguousarray(wd_full[rs]).astype(BF_NP),
                "wq_up": np.ascontiguousarray(wq_up[:, cs]).astype(BF_NP),
                "wv_up": np.ascontiguousarray(wv_up[:, cs]).astype(BF_NP),
                "kt": np.ascontiguousarray(kt_c).astype(F8_NP),
                "v": np.ascontiguousarray(v_c).astype(F8_NP),
                "wo": np.ascontiguousarray(wo[cs]).astype(BF_NP),
            }
        )
    return in_maps


def kernel(x, k_cache, v_cache, Wq_down, Wq_up, Wkv_down, Wk_up, Wv_up, Wo, **_):
    in_maps = make_in_maps(
        np.asarray(x), np.asarray(k_cache), np.asarray(v_cache),
        np.asarray(Wq_down), np.asarray(Wq_up), np.asarray(Wkv_down),
        np.asarray(Wv_up), np.asarray(Wo),
    )
    nc = _get_nc()
    res = bass_utils.run_bass_kernel_spmd(nc, in_maps, core_ids=list(range(NC_)))
    out = np.stack([res.results[b]["o"] for b in range(B)], axis=0)  # (8, 1, 7168)
    return np.ascontiguousarray(out, dtype=np.float32)


# revision 13
# speedup vs baseline: 1.1682x; 1.1682x over previous
"""DeepSeek-style MLA decode attention (batch=8, 128 heads, cache 512) on 8 NeuronCores.

Sharding: tensor-parallel over heads (16 heads/core).
 - Down-projection row-sharded over HID: core c computes a partial
   c = x_slice @ [Wq_down | Wkv_down]_slice; tiny AllReduce (64 KB) gives every
   core the full latent c = [c_q (1536) | c_kv (512)].
 - Wq_up / Wv_up column-sharded by head: each core computes q / v_new for its
   own 16 heads directly (no big collective on the q path).
 - k_cache / v_cache sharded by head, host-pretransposed, stored fp8e4m3
   (fp8 moving operands stream 2 elem/cycle on the PE).
 - Weights and matmul operands in bf16 (PSUM accumulation stays fp32).
 - o_proj input rows sharded by head; partial outputs ReduceScattered (bf16,
   3 overlapping chunks) over the batch dim; core b returns batch b's row.
 - Big DMA streams are split across both HW-DGE rings (sync + scalar
   engines) to get past the single-ring ~220 GB/s ceiling.

Note: the reference's "new token" softmax is over a length-1 axis (== 1.0), so
k_new/Wk_up are dead and the new-token contribution is simply + v_new.
"""

import numpy as np
import ml_dtypes

import concourse.bass as bass
import concourse.mybir as mybir
import concourse.tile as tile
from concourse import bacc
from concourse import bass_utils
from concourse.masks import make_identity

NC_ = 8                      # cores
B = 8                        # batch
H = 128                      # total heads
HP = H // NC_                # 16 heads per core
D = 128                      # head dim
L = 512                      # cache len
HID = 7168
CH = HID // NC_              # 896 hid rows per core (7 chunks of 128)
QL = 1536
KVL = 512
CL = QL + KVL                # 2048 latent dims
NH = HP * D                  # 2048 per-core head cols
SCALE = 1.0 / float(np.sqrt(D))
F32 = mybir.dt.float32
BF16 = mybir.dt.bfloat16
F8 = mybir.dt.float8e4

BF_NP = ml_dtypes.bfloat16
F8_NP = ml_dtypes.float8_e4m3

# o_proj rounds: (col0, col1, n accumulators of 512)
O_ROUNDS = ((0, 3072, 6), (3072, 6144, 6), (6144, 7168, 2))


def build_nc():
    nc = bacc.Bacc(
        "TRN2",
        target_bir_lowering=False,
        debug=False,
        enable_asserts=True,
        num_devices=NC_,
    )
    xt = nc.dram_tensor("xt", [CH, B], BF16, kind="ExternalInput").ap()
    w_down = nc.dram_tensor("w_down", [CH, CL], BF16, kind="ExternalInput").ap()
    wq_up = nc.dram_tensor("wq_up", [QL, NH], BF16, kind="ExternalInput").ap()
    wv_up = nc.dram_tensor("wv_up", [KVL, NH], BF16, kind="ExternalInput").ap()
    kt = nc.dram_tensor("kt", [8, 128, 8192], F8, kind="ExternalInput").ap()
    v = nc.dram_tensor("v", [8, 128, 8192], F8, kind="ExternalInput").ap()
    wo = nc.dram_tensor("wo", [NH, HID], BF16, kind="ExternalInput").ap()
    o = nc.dram_tensor("o", [1, HID], F32, kind="ExternalOutput").ap()

    rg = [list(range(NC_))]

    with tile.TileContext(nc) as tc:
        with (
            tc.tile_pool(name="const", bufs=1) as constp,
            tc.tile_pool(name="sbuf", bufs=1) as sb,
            tc.tile_pool(name="stage", bufs=2) as stg,
            tc.tile_pool(name="psbank", bufs=8, space="PSUM") as psbank,
            tc.tile_pool(name="dram", bufs=1, space="DRAM") as dram,
        ):
            ident = constp.tile([128, 128], F32)
            make_identity(nc, ident[:])
            id8 = ident[0:8, 0:8]
            # uint8 one-hot columns for CopyPredicated masks (must be int dtype)
            identu8 = constp.tile([128, 128], mybir.dt.uint8, tag="identu8")
            nc.vector.tensor_copy(identu8[:], ident[:])

            # ---------- partial latent: c_part = x_slice @ W_down_slice ----------
            xt_sb = constp.tile([128, 7, B], BF16, tag="xt")
            nc.sync.dma_start(
                out=xt_sb[:], in_=xt.rearrange("(c p) b -> p c b", p=128)
            )
            wd_sb = constp.tile([128, 7, CL], BF16, tag="wd")
            for i in range(7):
                nc.sync.dma_start(
                    out=wd_sb[:, i, :], in_=w_down[i * 128:(i + 1) * 128, :]
                )
            c_part = sb.tile([B, CL], F32, tag="cpart")
            ps_cds = [
                psbank.tile([B, 512], F32, tag="bank", name=f"ps_cd{n}")
                for n in range(4)
            ]
            for i in range(7):
                for n in range(4):
                    nc.tensor.matmul(
                        ps_cds[n][:B, :],
                        xt_sb[:, i, :],
                        wd_sb[:, i, n * 512:(n + 1) * 512],
                        start=(i == 0), stop=(i == 6),
                    )
            for n in range(4):
                nc.vector.tensor_copy(
                    c_part[:, n * 512:(n + 1) * 512], ps_cds[n][:B, :]
                )

            c_bounce = dram.tile([B, CL], F32, tag="cb")
            nc.sync.dma_start(out=c_bounce[:], in_=c_part[:])
            c_red = dram.tile([B, CL], F32, tag="cr")
            nc.gpsimd.collective_compute(
                "AllReduce",
                mybir.AluOpType.add,
                replica_groups=rg,
                ins=[c_bounce.opt()],
                outs=[c_red.opt()],
            )
            c_all = sb.tile([B, CL], F32, tag="call")
            nc.sync.dma_start(out=c_all[:], in_=c_red[:])

            # cT [128, 16*8]: rank-chunk j on partitions, batch on free
            ps_cT = psbank.tile([128, 512], F32, tag="bank")
            for j in range(16):
                nc.tensor.transpose(
                    ps_cT[0:128, j * 8:(j + 1) * 8],
                    c_all[:, j * 128:(j + 1) * 128],
                    id8,
                )
            cT = sb.tile([128, 128], BF16, tag="cT")
            nc.vector.tensor_copy(cT[:], ps_cT[:, 0:128])

            # ---------- q_own = c_q @ Wq_up_c ; vnew = c_kv @ Wv_up_c ----------
            wqup_sb = constp.tile([128, 12, NH], BF16, tag="wqup")
            for s in range(3):
                nc.sync.dma_start(
                    out=wqup_sb[:, s * 4:(s + 1) * 4, :],
                    in_=wq_up[s * 512:(s + 1) * 512, :].rearrange(
                        "(c p) n -> p c n", p=128
                    ),
                )
            wvup_sb = constp.tile([128, 4, NH], BF16, tag="wvup")
            nc.sync.dma_start(
                out=wvup_sb[:], in_=wv_up.rearrange("(c p) n -> p c n", p=128)
            )
            qown = sb.tile([B, NH], F32, tag="qown")
            vnew = sb.tile([B, NH], F32, tag="vnew")
            for n in range(4):
                ps_q = psbank.tile([B, 512], F32, tag="bank")
                for j in range(12):
                    nc.tensor.matmul(
                        ps_q[:B, :],
                        cT[:, j * 8:(j + 1) * 8],
                        wqup_sb[:, j, n * 512:(n + 1) * 512],
                        start=(j == 0), stop=(j == 11),
                    )
                nc.vector.tensor_copy(qown[:, n * 512:(n + 1) * 512], ps_q[:B, :])
                ps_vn = psbank.tile([B, 512], F32, tag="bank")
                for j in range(4):
                    nc.tensor.matmul(
                        ps_vn[:B, :],
                        cT[:, (12 + j) * 8:(13 + j) * 8],
                        wvup_sb[:, j, n * 512:(n + 1) * 512],
                        start=(j == 0), stop=(j == 3),
                    )
                nc.vector.tensor_copy(vnew[:, n * 512:(n + 1) * 512], ps_vn[:B, :])

            # qT [128 d, 128 hb] (hb = h*8+b), bf16 for the score matmuls
            ps_qT = psbank.tile([128, 512], F32, tag="bank")
            for h in range(HP):
                nc.tensor.transpose(
                    ps_qT[0:128, h * 8:(h + 1) * 8],
                    qown[:, h * D:(h + 1) * D],
                    id8,
                )
            qT = sb.tile([128, 128], BF16, tag="qT")
            nc.vector.tensor_copy(qT[:], ps_qT[:, 0:128])

            ktvp_ctx = tc.tile_pool(name="ktp", bufs=3)
            ktp = ktvp_ctx.__enter__()
            vp_ctx = tc.tile_pool(name="vp", bufs=2)
            vp = vp_ctx.__enter__()
            # ---------------- phase A: scores over k cache ----------------
            # lhsT = qT (bf16, stationary); rhs = fp8 kT tile (moving, N=512).
            # Out row hb of each product is the valid score row; extract it
            # (bf16) with a partition-aligned predicated copy on the DVE.
            scores = sb.tile([128, 512], BF16, tag="scores")
            for t8 in range(8):
                kt_t = ktp.tile([128, 8192], F8, tag="kt")
                nc.scalar.dma_start(out=kt_t[:], in_=kt[t8])
                for u in range(16):
                    hb = 16 * t8 + u
                    ps_s = psbank.tile([128, 512], F32, tag="bank")
                    nc.tensor.matmul(
                        ps_s[:], qT[:], kt_t[:, u * 512:(u + 1) * 512],
                        start=True, stop=True,
                    )
                    # write only row hb (engines can't address partition hb
                    # directly: start partition must be 0/32/64/96)
                    nc.vector.copy_predicated(
                        scores[:],
                        identu8[:, hb:hb + 1].broadcast_to((128, 512)),
                        ps_s[:],
                    )

            probs = sb.tile([128, 512], F32, tag="probs")
            denom = sb.tile([128, 1], F32, tag="denom")
            nc.scalar.activation(
                probs[:], scores[:], mybir.ActivationFunctionType.Exp,
                scale=SCALE, accum_out=denom[:],
            )
            recip = sb.tile([128, 1], F32, tag="recip")
            nc.vector.reciprocal(recip[:], denom[:])
            probsn = sb.tile([128, 512], F32, tag="probsn")
            nc.vector.tensor_scalar_mul(probsn[:], probs[:], recip[:])

            ps_pT = psbank.tile([128, 512], F32, tag="bank")
            for cc in range(4):
                nc.tensor.transpose(
                    ps_pT[:, cc * 128:(cc + 1) * 128],
                    probsn[:, cc * 128:(cc + 1) * 128],
                    ident[:],
                )
            probsT = sb.tile([128, 4, 128], BF16, tag="probsT")
            nc.vector.tensor_copy(
                probsT[:].rearrange("p c n -> p (c n)"), ps_pT[:]
            )

            # ---------------- phase B: attn rows = probs @ V ----------------
            # Per group of 4 hb: lhsT = probsT chunk cc (bf16, all hb), rhs
            # packs the 4 hb's fp8 V chunk cc; accumulate over cc, then
            # extract row 4g+u from column block u.
            attn = sb.tile([128, 128], F32, tag="attn")
            for g8 in range(8):
                v_t = vp.tile([128, 8192], F8, tag="v")
                nc.sync.dma_start(out=v_t[:], in_=v[g8])
                for gg in range(4):
                    g = 4 * g8 + gg
                    ps_a = psbank.tile([128, 512], F32, tag="bank")
                    for cc in range(4):
                        nc.tensor.matmul(
                            ps_a[:],
                            probsT[:, cc, :],
                            v_t[:, gg * 2048 + cc * 512:gg * 2048 + (cc + 1) * 512],
                            start=(cc == 0), stop=(cc == 3),
                        )
                    for u in range(4):
                        hb = 4 * g + u
                        nc.vector.copy_predicated(
                            attn[:],
                            identu8[:, hb:hb + 1].broadcast_to((128, 128)),
                            ps_a[:, u * 128:(u + 1) * 128],
                        )

            # vnewT, then attnT = attn^T + vnewT (bf16 operand for o_proj)
            ps_vT = psbank.tile([128, 512], F32, tag="bank")
            for h in range(HP):
                nc.tensor.transpose(
                    ps_vT[0:128, h * 8:(h + 1) * 8],
                    vnew[:, h * D:(h + 1) * D],
                    id8,
                )
            vnewT = sb.tile([128, 128], F32, tag="vnewT")
            nc.vector.tensor_copy(vnewT[:], ps_vT[:, 0:128])
            ps_aT = psbank.tile([128, 512], F32, tag="bank")
            nc.tensor.transpose(ps_aT[:, 0:128], attn[:], ident[:])
            attnT = sb.tile([128, 128], BF16, tag="attnT")
            nc.vector.tensor_add(attnT[:], ps_aT[:, 0:128], vnewT[:])

            vp_ctx.__exit__(None, None, None)
            ktvp_ctx.__exit__(None, None, None)
            wop_ctx = tc.tile_pool(name="wop", bufs=5)
            wop = wop_ctx.__enter__()
            # ---------------- phase C: o_part = attn^T @ Wo_c ----------------
            # Rounds of up to 6 n-chunks; Wo streams as per-head row blocks.
            # Partial outputs RS'd (bf16) per round, overlapping the next round.
            ors_parts = []
            for r, (n0, n1, nn) in enumerate(O_ROUNDS):
                ps_os = [
                    psbank.tile([B, 512], F32, tag="bank", name=f"ps_o{r}_{i}")
                    for i in range(nn)
                ]
                for h in range(HP):
                    wo_t = wop.tile([128, 3072], BF16, tag="wo")
                    eng = nc.sync if h % 2 == 0 else nc.scalar
                    eng.dma_start(
                        out=wo_t[:, 0:n1 - n0],
                        in_=wo[h * D:(h + 1) * D, n0:n1],
                    )
                    for i in range(nn):
                        nc.tensor.matmul(
                            ps_os[i][:B, :],
                            attnT[:, h * 8:(h + 1) * 8],
                            wo_t[:, i * 512:(i + 1) * 512],
                            start=(h == 0), stop=(h == HP - 1),
                        )
                ostage = stg.tile([B, 3072], BF16, tag="ostage")
                for i in range(nn):
                    nc.vector.tensor_copy(
                        ostage[:, i * 512:(i + 1) * 512], ps_os[i][:B, :]
                    )
                ob_r = dram.tile([B, n1 - n0], BF16, tag=f"ob{r}", name=f"ob{r}")
                nc.sync.dma_start(out=ob_r[:], in_=ostage[:, 0:n1 - n0])
                or_r = dram.tile([1, n1 - n0], BF16, tag=f"or{r}", name=f"or{r}")
                nc.gpsimd.collective_compute(
                    "ReduceScatter",
                    mybir.AluOpType.add,
                    replica_groups=rg,
                    ins=[ob_r.opt()],
                    outs=[or_r.opt()],
                )
                ors_parts.append((or_r, n0, n1))

            for or_r, n0, n1 in ors_parts:
                fin_b = stg.tile([1, 3072], BF16, tag="finb", bufs=1)
                nc.sync.dma_start(out=fin_b[:, 0:n1 - n0], in_=or_r[:])
                for f0 in range(0, n1 - n0, 1536):
                    f1 = min(f0 + 1536, n1 - n0)
                    fin_f = stg.tile([1, 1536], F32, tag="finf", bufs=1)
                    nc.vector.tensor_copy(fin_f[:, 0:f1 - f0], fin_b[:, f0:f1])
                    nc.sync.dma_start(
                        out=o[:, n0 + f0:n0 + f1], in_=fin_f[:, 0:f1 - f0]
                    )
            wop_ctx.__exit__(None, None, None)

    nc.compile()
    return nc


_NC_CACHE = None


def _get_nc():
    global _NC_CACHE
    if _NC_CACHE is None:
        _NC_CACHE = build_nc()
    return _NC_CACHE


def make_in_maps(x, k_cache, v_cache, Wq_down, Wq_up, Wkv_down, Wv_up, Wo):
    xT = np.ascontiguousarray(np.asarray(x, np.float32).reshape(B, HID).T)
    wd_full = np.concatenate(
        [np.asarray(Wq_down, np.float32), np.asarray(Wkv_down, np.float32)], axis=1
    )
    k = np.asarray(k_cache, np.float32)
    v = np.asarray(v_cache, np.float32)
    wq_up = np.asarray(Wq_up, np.float32)
    wv_up = np.asarray(Wv_up, np.float32)
    wo = np.asarray(Wo, np.float32)
    in_maps = []
    for c in range(NC_):
        hs = slice(c * HP, (c + 1) * HP)
        rs = slice(c * CH, (c + 1) * CH)
        cs = slice(c * NH, (c + 1) * NH)
        # kt merged tile t8 holds hb=16*t8..16*t8+16 as [128 d, (u, keys)]
        kt_c = (
            k[:, hs]
            .transpose(1, 0, 3, 2)          # (16, 8, 128, 512) [h, b, d, l]
            .reshape(32, 4, 128, 512)       # [g, t, d, l]
            .transpose(0, 2, 1, 3)          # [g, d, t, l]
            .reshape(8, 4, 128, 2048)       # [t8, tt, d, (t l)]
            .transpose(0, 2, 1, 3)          # [t8, d, tt, (t l)]
            .reshape(8, 128, 8192)
        )
        # v merged tile g8 holds hb=16*g8..16*g8+16 as [128 lp, (gg, cc, t, d)]
        v_c = (
            v[:, hs]
            .transpose(1, 0, 2, 3)          # (16, 8, 512, 128) [h, b, l, d]
            .reshape(32, 4, 4, 128, 128)    # [g, t, cc, lp, d]
            .transpose(0, 3, 2, 1, 4)       # [g, lp, cc, t, d]
            .reshape(8, 4, 128, 2048)       # [g8, gg, lp, (cc t d)]
            .transpose(0, 2, 1, 3)          # [g8, lp, gg, (cc t d)]
            .reshape(8, 128, 8192)
        )
        in_maps.append(
            {
                "xt": np.ascontiguousarray(xT[rs]).astype(BF_NP),
                "w_down": np.ascontiguousarray(wd_full[rs]).astype(BF_NP),
                "wq_up": np.ascontiguousarray(wq_up[:, cs]).astype(BF_NP),
                "wv_up": np.ascontiguousarray(wv_up[:, cs]).astype(BF_NP),
                "kt": np.ascontiguousarray(kt_c).astype(F8_NP),
                "v": np.ascontiguousarray(v_c).astype(F8_NP),
                "wo": np.ascontiguousarray(wo[cs]).astype(BF_NP),
            }
        )
    return in_maps


def kernel(x, k_cache, v_cache, Wq_down, Wq_up, Wkv_down, Wk_up, Wv_up, Wo, **_):
    in_maps = make_in_maps(
        np.asarray(x), np.asarray(k_cache), np.asarray(v_cache),
        np.asarray(Wq_down), np.asarray(Wq_up), np.asarray(Wkv_down),
        np.asarray(Wv_up), np.asarray(Wo),
    )
    nc = _get_nc()
    res = bass_utils.run_bass_kernel_spmd(nc, in_maps, core_ids=list(range(NC_)))
    out = np.stack([res.results[b]["o"] for b in range(B)], axis=0)  # (8, 1, 7168)
    return np.ascontiguousarray(out, dtype=np.float32)


# revision 14
# speedup vs baseline: 1.2586x; 1.0774x over previous
"""DeepSeek-style MLA decode attention (batch=8, 128 heads, cache 512) on 8 NeuronCores.

Sharding: tensor-parallel over heads (16 heads/core).
 - Down-projection row-sharded over HID: core c computes a partial
   c = x_slice @ [Wq_down | Wkv_down]_slice; tiny AllReduce (64 KB) gives every
   core the full latent c = [c_q (1536) | c_kv (512)].
 - Wq_up / Wv_up column-sharded by head: each core computes q / v_new for its
   own 16 heads directly (no big collective on the q path).
 - k_cache / v_cache sharded by head, host-pretransposed, stored fp8e4m3
   (fp8 moving operands stream 2 elem/cycle on the PE).
 - Weights and matmul operands in bf16 (PSUM accumulation stays fp32).
 - o_proj input rows sharded by head; partial outputs ReduceScattered (bf16,
   3 overlapping chunks) over the batch dim; core b returns batch b's row.
 - Big DMA streams are split across both HW-DGE rings (sync + scalar
   engines) to get past the single-ring ~220 GB/s ceiling.

Note: the reference's "new token" softmax is over a length-1 axis (== 1.0), so
k_new/Wk_up are dead and the new-token contribution is simply + v_new.
"""

import numpy as np
import ml_dtypes

import concourse.bass as bass
import concourse.mybir as mybir
import concourse.tile as tile
from concourse import bacc
from concourse import bass_utils
from concourse.masks import make_identity

NC_ = 8                      # cores
B = 8                        # batch
H = 128                      # total heads
HP = H // NC_                # 16 heads per core
D = 128                      # head dim
L = 512                      # cache len
HID = 7168
CH = HID // NC_              # 896 hid rows per core (7 chunks of 128)
QL = 1536
KVL = 512
CL = QL + KVL                # 2048 latent dims
NH = HP * D                  # 2048 per-core head cols
SCALE = 1.0 / float(np.sqrt(D))
F32 = mybir.dt.float32
BF16 = mybir.dt.bfloat16
F8 = mybir.dt.float8e4

BF_NP = ml_dtypes.bfloat16
F8_NP = ml_dtypes.float8_e4m3

# o_proj rounds: (col0, col1, n accumulators of 512)
O_ROUNDS = ((0, 3072, 6), (3072, 6144, 6), (6144, 7168, 2))


def build_nc():
    nc = bacc.Bacc(
        "TRN2",
        target_bir_lowering=False,
        debug=False,
        enable_asserts=True,
        num_devices=NC_,
    )
    xt = nc.dram_tensor("xt", [CH, B], BF16, kind="ExternalInput").ap()
    w_down = nc.dram_tensor("w_down", [CH, CL], BF16, kind="ExternalInput").ap()
    wq_up = nc.dram_tensor("wq_up", [QL, NH], BF16, kind="ExternalInput").ap()
    wv_up = nc.dram_tensor("wv_up", [KVL, NH], BF16, kind="ExternalInput").ap()
    kt = nc.dram_tensor("kt", [8, 128, 8192], F8, kind="ExternalInput").ap()
    v = nc.dram_tensor("v", [8, 128, 8192], F8, kind="ExternalInput").ap()
    wo = nc.dram_tensor("wo", [NH, HID], BF16, kind="ExternalInput").ap()
    o = nc.dram_tensor("o", [1, HID], F32, kind="ExternalOutput").ap()

    rg = [list(range(NC_))]

    with tile.TileContext(nc) as tc:
        with (
            tc.tile_pool(name="const", bufs=1) as constp,
            tc.tile_pool(name="sbuf", bufs=1) as sb,
            tc.tile_pool(name="stage", bufs=2) as stg,
            tc.tile_pool(name="dram", bufs=1, space="DRAM") as dram,
        ):
            ident = constp.tile([128, 128], F32)
            make_identity(nc, ident[:])
            id8 = ident[0:8, 0:8]
            # uint8 one-hot columns for CopyPredicated masks (must be int dtype)
            identu8 = constp.tile([128, 128], mybir.dt.uint8, tag="identu8")
            nc.vector.tensor_copy(identu8[:], ident[:])

            psA_ctx = tc.tile_pool(name="psA", bufs=6, space="PSUM")
            psbank = psA_ctx.__enter__()
            # ---------- partial latent: c_part = x_slice @ W_down_slice ----------
            xt_sb = constp.tile([128, 7, B], BF16, tag="xt")
            nc.sync.dma_start(
                out=xt_sb[:], in_=xt.rearrange("(c p) b -> p c b", p=128)
            )
            wd_sb = constp.tile([128, 7, CL], BF16, tag="wd")
            for i in range(7):
                nc.sync.dma_start(
                    out=wd_sb[:, i, :], in_=w_down[i * 128:(i + 1) * 128, :]
                )
            c_part = sb.tile([B, CL], F32, tag="cpart")
            ps_cds = [
                psbank.tile([B, 512], F32, tag="bank", name=f"ps_cd{n}")
                for n in range(4)
            ]
            for i in range(7):
                for n in range(4):
                    nc.tensor.matmul(
                        ps_cds[n][:B, :],
                        xt_sb[:, i, :],
                        wd_sb[:, i, n * 512:(n + 1) * 512],
                        start=(i == 0), stop=(i == 6),
                    )
            for n in range(4):
                nc.vector.tensor_copy(
                    c_part[:, n * 512:(n + 1) * 512], ps_cds[n][:B, :]
                )

            c_bounce = dram.tile([B, CL], F32, tag="cb")
            nc.sync.dma_start(out=c_bounce[:], in_=c_part[:])
            c_red = dram.tile([B, CL], F32, tag="cr")
            nc.gpsimd.collective_compute(
                "AllReduce",
                mybir.AluOpType.add,
                replica_groups=rg,
                ins=[c_bounce.opt()],
                outs=[c_red.opt()],
            )
            c_all = sb.tile([B, CL], F32, tag="call")
            nc.sync.dma_start(out=c_all[:], in_=c_red[:])

            # cT [128, 16*8]: rank-chunk j on partitions, batch on free
            ps_cT = psbank.tile([128, 512], F32, tag="bank")
            for j in range(16):
                nc.tensor.transpose(
                    ps_cT[0:128, j * 8:(j + 1) * 8],
                    c_all[:, j * 128:(j + 1) * 128],
                    id8,
                )
            cT = sb.tile([128, 128], BF16, tag="cT")
            nc.vector.tensor_copy(cT[:], ps_cT[:, 0:128])

            # ---------- q_own = c_q @ Wq_up_c ; vnew = c_kv @ Wv_up_c ----------
            wqup_sb = constp.tile([128, 12, NH], BF16, tag="wqup")
            for s in range(3):
                nc.sync.dma_start(
                    out=wqup_sb[:, s * 4:(s + 1) * 4, :],
                    in_=wq_up[s * 512:(s + 1) * 512, :].rearrange(
                        "(c p) n -> p c n", p=128
                    ),
                )
            wvup_sb = constp.tile([128, 4, NH], BF16, tag="wvup")
            nc.sync.dma_start(
                out=wvup_sb[:], in_=wv_up.rearrange("(c p) n -> p c n", p=128)
            )
            qown = sb.tile([B, NH], F32, tag="qown")
            vnew = sb.tile([B, NH], F32, tag="vnew")
            for n in range(4):
                ps_q = psbank.tile([B, 512], F32, tag="bank")
                for j in range(12):
                    nc.tensor.matmul(
                        ps_q[:B, :],
                        cT[:, j * 8:(j + 1) * 8],
                        wqup_sb[:, j, n * 512:(n + 1) * 512],
                        start=(j == 0), stop=(j == 11),
                    )
                nc.vector.tensor_copy(qown[:, n * 512:(n + 1) * 512], ps_q[:B, :])
                ps_vn = psbank.tile([B, 512], F32, tag="bank")
                for j in range(4):
                    nc.tensor.matmul(
                        ps_vn[:B, :],
                        cT[:, (12 + j) * 8:(13 + j) * 8],
                        wvup_sb[:, j, n * 512:(n + 1) * 512],
                        start=(j == 0), stop=(j == 3),
                    )
                nc.vector.tensor_copy(vnew[:, n * 512:(n + 1) * 512], ps_vn[:B, :])

            # qT [128 d, 128 hb] (hb = h*8+b), bf16 for the score matmuls
            ps_qT = psbank.tile([128, 512], F32, tag="bank")
            for h in range(HP):
                nc.tensor.transpose(
                    ps_qT[0:128, h * 8:(h + 1) * 8],
                    qown[:, h * D:(h + 1) * D],
                    id8,
                )
            qT = sb.tile([128, 128], BF16, tag="qT")
            nc.vector.tensor_copy(qT[:], ps_qT[:, 0:128])

            psA_ctx.__exit__(None, None, None)
            ktvp_ctx = tc.tile_pool(name="ktp", bufs=3)
            ktp = ktvp_ctx.__enter__()
            vp_ctx = tc.tile_pool(name="vp", bufs=2)
            vp = vp_ctx.__enter__()
            psQ_ctx = tc.tile_pool(name="psQ", bufs=2, space="PSUM")
            psQ = psQ_ctx.__enter__()
            # ---------------- phase A: scores over k cache ----------------
            # lhsT = qT (bf16, stationary); rhs = fp8 kT tile (moving, N=512).
            # 4 matmuls fill a 4-bank PSUM quad; ONE predicated copy with a
            # stride-0-broadcast out AP extracts all 4 valid rows at once.
            scores = sb.tile([128, 512], BF16, tag="scores")
            for t8 in range(8):
                kt_t = ktp.tile([128, 8192], F8, tag="kt")
                nc.scalar.dma_start(out=kt_t[:], in_=kt[t8])
                for q4 in range(4):
                    ps_s = psQ.tile([128, 4, 512], F32, tag="quad")
                    for uu in range(4):
                        u = 4 * q4 + uu
                        nc.tensor.matmul(
                            ps_s[:, uu, :], qT[:], kt_t[:, u * 512:(u + 1) * 512],
                            start=True, stop=True,
                        )
                    hb0 = 16 * t8 + 4 * q4
                    nc.vector.copy_predicated(
                        scores[:].rearrange("p (o n) -> p o n", o=1).broadcast_to(
                            (128, 4, 512)
                        ),
                        identu8[:, hb0:hb0 + 4].rearrange(
                            "p (c o) -> p c o", o=1
                        ).broadcast_to((128, 4, 512)),
                        ps_s[:],
                    )

            # vnew^T in the PE shadow of the score extraction drain
            ps_vq = psQ.tile([128, 4, 512], F32, tag="quad")
            for h in range(HP):
                nc.tensor.transpose(
                    ps_vq[0:128, 0, h * 8:(h + 1) * 8],
                    vnew[:, h * D:(h + 1) * D],
                    id8,
                )
            vnewT = sb.tile([128, 128], F32, tag="vnewT")
            nc.vector.tensor_copy(vnewT[:], ps_vq[:, 0, 0:128])
            psQ_ctx.__exit__(None, None, None)
            psB_ctx = tc.tile_pool(name="psB", bufs=6, space="PSUM")
            psbank = psB_ctx.__enter__()

            probs = sb.tile([128, 512], F32, tag="probs")
            denom = sb.tile([128, 1], F32, tag="denom")
            nc.scalar.activation(
                probs[:], scores[:], mybir.ActivationFunctionType.Exp,
                scale=SCALE, accum_out=denom[:],
            )
            recip = sb.tile([128, 1], F32, tag="recip")
            nc.vector.reciprocal(recip[:], denom[:])
            probsn = sb.tile([128, 512], F32, tag="probsn")
            nc.vector.tensor_scalar_mul(probsn[:], probs[:], recip[:])

            ps_pT = psbank.tile([128, 512], F32, tag="bank")
            for cc in range(4):
                nc.tensor.transpose(
                    ps_pT[:, cc * 128:(cc + 1) * 128],
                    probsn[:, cc * 128:(cc + 1) * 128],
                    ident[:],
                )
            probsT = sb.tile([128, 4, 128], BF16, tag="probsT")
            nc.vector.tensor_copy(
                probsT[:].rearrange("p c n -> p (c n)"), ps_pT[:]
            )

            # ---------------- phase B: attn rows = probs @ V ----------------
            # Per group of 4 hb: lhsT = probsT chunk cc (bf16, all hb), rhs
            # packs the 4 hb's fp8 V chunk cc; accumulate over cc, then
            # extract row 4g+u from column block u.
            attn = sb.tile([128, 128], F32, tag="attn")
            for g8 in range(8):
                v_t = vp.tile([128, 8192], F8, tag="v")
                nc.sync.dma_start(out=v_t[:], in_=v[g8])
                for gg in range(4):
                    g = 4 * g8 + gg
                    ps_a = psbank.tile([128, 512], F32, tag="bank")
                    for cc in range(4):
                        nc.tensor.matmul(
                            ps_a[:],
                            probsT[:, cc, :],
                            v_t[:, gg * 2048 + cc * 512:gg * 2048 + (cc + 1) * 512],
                            start=(cc == 0), stop=(cc == 3),
                        )
                    nc.vector.copy_predicated(
                        attn[:].rearrange("p (o n) -> p o n", o=1).broadcast_to(
                            (128, 4, 128)
                        ),
                        identu8[:, 4 * g:4 * g + 4].rearrange(
                            "p (c o) -> p c o", o=1
                        ).broadcast_to((128, 4, 128)),
                        ps_a[:].rearrange("p (c n) -> p c n", c=4),
                    )

            # attnT = attn^T + vnewT (bf16 operand for o_proj)
            ps_aT = psbank.tile([128, 512], F32, tag="bank")
            nc.tensor.transpose(ps_aT[:, 0:128], attn[:], ident[:])
            attnT = sb.tile([128, 128], BF16, tag="attnT")
            nc.vector.tensor_add(attnT[:], ps_aT[:, 0:128], vnewT[:])

            vp_ctx.__exit__(None, None, None)
            ktvp_ctx.__exit__(None, None, None)
            wop_ctx = tc.tile_pool(name="wop", bufs=5)
            wop = wop_ctx.__enter__()
            # ---------------- phase C: o_part = attn^T @ Wo_c ----------------
            # Rounds of up to 6 n-chunks; Wo streams as per-head row blocks.
            # Partial outputs RS'd (bf16) per round, overlapping the next round.
            ors_parts = []
            for r, (n0, n1, nn) in enumerate(O_ROUNDS):
                ps_os = [
                    psbank.tile([B, 512], F32, tag="bank", name=f"ps_o{r}_{i}")
                    for i in range(nn)
                ]
                for h in range(HP):
                    wo_t = wop.tile([128, 3072], BF16, tag="wo")
                    eng = nc.sync if h % 2 == 0 else nc.scalar
                    eng.dma_start(
                        out=wo_t[:, 0:n1 - n0],
                        in_=wo[h * D:(h + 1) * D, n0:n1],
                    )
                    for i in range(nn):
                        nc.tensor.matmul(
                            ps_os[i][:B, :],
                            attnT[:, h * 8:(h + 1) * 8],
                            wo_t[:, i * 512:(i + 1) * 512],
                            start=(h == 0), stop=(h == HP - 1),
                        )
                ostage = stg.tile([B, 3072], BF16, tag="ostage")
                for i in range(nn):
                    nc.vector.tensor_copy(
                        ostage[:, i * 512:(i + 1) * 512], ps_os[i][:B, :]
                    )
                ob_r = dram.tile([B, n1 - n0], BF16, tag=f"ob{r}", name=f"ob{r}")
                nc.sync.dma_start(out=ob_r[:], in_=ostage[:, 0:n1 - n0])
                or_r = dram.tile([1, n1 - n0], BF16, tag=f"or{r}", name=f"or{r}")
                nc.gpsimd.collective_compute(
                    "ReduceScatter",
                    mybir.AluOpType.add,
                    replica_groups=rg,
                    ins=[ob_r.opt()],
                    outs=[or_r.opt()],
                )
                ors_parts.append((or_r, n0, n1))
                or_r, n0, n1 = ors_parts[-1]
                fin_b = stg.tile([1, 3072], BF16, tag="finb", bufs=1)
                nc.sync.dma_start(out=fin_b[:, 0:n1 - n0], in_=or_r[:])
                for f0 in range(0, n1 - n0, 1536):
                    f1 = min(f0 + 1536, n1 - n0)
                    fin_f = stg.tile([1, 1536], F32, tag="finf", bufs=1)
                    nc.vector.tensor_copy(fin_f[:, 0:f1 - f0], fin_b[:, f0:f1])
                    nc.sync.dma_start(
                        out=o[:, n0 + f0:n0 + f1], in_=fin_f[:, 0:f1 - f0]
                    )
            wop_ctx.__exit__(None, None, None)
            psB_ctx.__exit__(None, None, None)

    nc.compile()
    return nc


_NC_CACHE = None


def _get_nc():
    global _NC_CACHE
    if _NC_CACHE is None:
        _NC_CACHE = build_nc()
    return _NC_CACHE


def make_in_maps(x, k_cache, v_cache, Wq_down, Wq_up, Wkv_down, Wv_up, Wo):
    xT = np.ascontiguousarray(np.asarray(x, np.float32).reshape(B, HID).T)
    wd_full = np.concatenate(
        [np.asarray(Wq_down, np.float32), np.asarray(Wkv_down, np.float32)], axis=1
    )
    k = np.asarray(k_cache, np.float32)
    v = np.asarray(v_cache, np.float32)
    wq_up = np.asarray(Wq_up, np.float32)
    wv_up = np.asarray(Wv_up, np.float32)
    wo = np.asarray(Wo, np.float32)
    in_maps = []
    for c in range(NC_):
        hs = slice(c * HP, (c + 1) * HP)
        rs = slice(c * CH, (c + 1) * CH)
        cs = slice(c * NH, (c + 1) * NH)
        # kt merged tile t8 holds hb=16*t8..16*t8+16 as [128 d, (u, keys)]
        kt_c = (
            k[:, hs]
            .transpose(1, 0, 3, 2)          # (16, 8, 128, 512) [h, b, d, l]
            .reshape(32, 4, 128, 512)       # [g, t, d, l]
            .transpose(0, 2, 1, 3)          # [g, d, t, l]
            .reshape(8, 4, 128, 2048)       # [t8, tt, d, (t l)]
            .transpose(0, 2, 1, 3)          # [t8, d, tt, (t l)]
            .reshape(8, 128, 8192)
        )
        # v merged tile g8 holds hb=16*g8..16*g8+16 as [128 lp, (gg, cc, t, d)]
        v_c = (
            v[:, hs]
            .transpose(1, 0, 2, 3)          # (16, 8, 512, 128) [h, b, l, d]
            .reshape(32, 4, 4, 128, 128)    # [g, t, cc, lp, d]
            .transpose(0, 3, 2, 1, 4)       # [g, lp, cc, t, d]
            .reshape(8, 4, 128, 2048)       # [g8, gg, lp, (cc t d)]
            .transpose(0, 2, 1, 3)          # [g8, lp, gg, (cc t d)]
            .reshape(8, 128, 8192)
        )
        in_maps.append(
            {
                "xt": np.ascontiguousarray(xT[rs]).astype(BF_NP),
                "w_down": np.ascontiguousarray(wd_full[rs]).astype(BF_NP),
                "wq_up": np.ascontiguousarray(wq_up[:, cs]).astype(BF_NP),
                "wv_up": np.ascontiguousarray(wv_up[:, cs]).astype(BF_NP),
                "kt": np.ascontiguousarray(kt_c).astype(F8_NP),
                "v": np.ascontiguousarray(v_c).astype(F8_NP),
                "wo": np.ascontiguousarray(wo[cs]).astype(BF_NP),
            }
        )
    return in_maps


def kernel(x, k_cache, v_cache, Wq_down, Wq_up, Wkv_down, Wk_up, Wv_up, Wo, **_):
    in_maps = make_in_maps(
        np.asarray(x), np.asarray(k_cache), np.asarray(v_cache),
        np.asarray(Wq_down), np.asarray(Wq_up), np.asarray(Wkv_down),
        np.asarray(Wv_up), np.asarray(Wo),
    )
    nc = _get_nc()
    res = bass_utils.run_bass_kernel_spmd(nc, in_maps, core_ids=list(range(NC_)))
    out = np.stack([res.results[b]["o"] for b in range(B)], axis=0)  # (8, 1, 7168)
    return np.ascontiguousarray(out, dtype=np.float32)
